# revision 1
# baseline (speedup 1.0000x reference)
"""Trainium2 Bass kernel for nn_Amplified_PatternMixer.

Computation:
  out[b, h, m1, m2] = mixed_pattern[h, m1, m2] + alpha[h] * nrm[b, m2]
where
  nrm[b, m] = || mean_{hw}(x[b*57+m, :, h, w]) ||_2   over channels
  mixed_pattern = tiny 57x57 graph-normalized pattern (from 5x7x7 params).

The memory-bound part (streaming x: [1824, 256, 14, 14] f32, ~366 MB) runs
on 8 NeuronCores, data-parallel over rows (228 rows/core).

Per-core layout: each row (256 ch x 196 hw) is split into 4 quarter-rows of
64 channels; the 912 quarter-rows tile as 7 x [128 part, 64ch*196] plus one
[128 part, 8ch*196] tail (rows 224..227 split 32-ways).  Every tile's HBM
source is one fully contiguous range fanned over all 128 partitions, so all
16 SDMA engines carry identical byte loads (57 * 50176 B each).

Loads are issued via HWDGE (nc.sync/nc.scalar) rather than SWDGE
(nc.gpsimd): SWDGE descriptor generation runs on the GpSimd Q7 cores, which
are locked out of the shared SBUF port while DVE reduce ops run - the
baseline trace showed per-engine DMA rate collapsing from 27.1 GB/s to
~21 GB/s whenever vector was active.  HWDGE descriptors are generated in
RTL and are immune.

Each tile: vector reduce over hw=196 -> per-channel sums cs[128, w];
scalar Square-activation with accum_out -> per-piece sum of squares
(one partial per partition per tile).  Host combines the 4 (or 32)
partials per row, sqrt, /196.  The tiny 57x57 pattern-mixer runs on host.
"""

import numpy as np

import concourse.bacc as bacc
import concourse.mybir as mybir
import concourse.tile as tile
from concourse.bass_utils import run_bass_kernel_spmd

# Problem constants (hardcoded; kernel.py must be self-contained).
NUM_BASIC = 5
NUM_MIXED = 4
NUM_FRAME = 8
NUM_NODES = 7
NUM_SAMPLES = 8
M = 1 + NUM_NODES * NUM_FRAME  # 57

N_CORES = 8
B = 32
C = 256
HW = 196  # 14*14
ROWS_TOTAL = B * M          # 1824
ROWS_PER_CORE = ROWS_TOTAL // N_CORES  # 228
CW = C * HW                 # 50176 floats per row

# (row_start, w): tile covers rows row_start..row_start+w//2, as 128 pieces
# of w channels each (w/2 rows x 256/w pieces).  Per-partition run is
# w*196*4 bytes (<= 64KB keeps one descriptor per partition).
# Graded ramp + uniform 32ch body + tapered tail.  The ramp lets vector
# start ~1us after the first DMA lands; 32ch tiles hit the DVE reduce
# sweet spot (32-wide groups: 1.066 ns/elem vs 1.26 at 64-wide) and per
# tile vector work (6.7us) stays under DMA time (7.6us) so the pipeline
# is DMA-paced; deep buffering (bufs=4) keeps DMA issue from ever gating
# on vector; the taper keeps the post-DMA drain ~3us.  Best measured:
# 130.1us (quiet HBM); shared-HBM neighbor bursts add +10..25us to any
# config.  The 64ch-body + gpsimd-fold variant measured 133.0-133.4 in
# the same windows.
_WIDTHS = [4, 8, 16, 32] + [32] * 11 + [16, 16, 8, 4]
assert sum(_WIDTHS) == 2 * ROWS_PER_CORE
TILE_PLAN = []
_r = 0
for _w in _WIDTHS:
    TILE_PLAN.append((_r, _w))
    _r += _w // 2
N_TILES = len(TILE_PLAN)

# Per-tile GpSimd fold policy: number of channels whose hw-reduction is
# pre-halved (196 -> 98) by gpsimd.tensor_add before the vector reduce.
# Body 64-tiles fold half (raw reduce runs parallel to the fold); tail
# tiles fold (half for w=32, fully for w<=16) since gpsimd is idle there
# and vector is the drain path; the final tile stays raw (shortest
# latency chain).
def _fold_plan(widths):
    # With the 32ch body the vector engine keeps pace on raw reduces
    # (~95us total vs ~111us of DMA); gpsimd folding is not needed.
    return [0 for _ in widths]

_FOLDS = _fold_plan(_WIDTHS)

LAST_RESULT = None
_NC_CACHE = None


def _build_nc_hwdge(plan=TILE_PLAN, folds=_FOLDS, bufs=4,
                    rings=("sync", "scalar"), scalar_sq=True,
                    scalar_tiles=()):
    """HWDGE row-piece kernel.

    rings: cycle of engine names for the tile load DMAs (HWDGE engines:
    sync, scalar); alternating measured best.  Outstanding HWDGE DMAs
    time-share the SDMA engines (even on one queue), so completion sems
    fire ~(in-flight-behind bytes)/BW late -- another reason to keep
    tiles moderate and buffering deep.
    folds[t]: optional GpSimd pre-fold of the last folds[t] channels
    hw 196 -> 98 (tensor_add on its own SBUF door) to offload vector;
    unused in the default 32ch-body plan (vector keeps pace raw).
    bufs=4 ensures DMA issue (gated by consumer-done of the tile 4
    back) never waits on a vector hiccup.
    """
    nc = bacc.Bacc(None)
    x = nc.declare_dram_parameter(
        "x", [ROWS_PER_CORE, CW], mybir.dt.float32, isOutput=False
    )
    out = nc.declare_dram_parameter(
        "out", [128, len(plan)], mybir.dt.float32, isOutput=True
    )
    max_w = max(w for _, w in plan)
    max_fold = max(folds) if folds else 0
    scalar_tiles = scalar_tiles or ()
    fw = HW // 2  # 98
    with tile.TileContext(nc) as tc:
        with (
            tc.tile_pool(name="xt_pool", bufs=bufs) as xp,
            tc.tile_pool(name="fold_pool", bufs=2) as fp,
            tc.tile_pool(name="acc_pool", bufs=2) as accp,
            tc.tile_pool(name="res_pool", bufs=1) as resp,
            tc.tile_pool(name="trash_pool", bufs=2) as trp,
        ):
            osb = resp.tile([128, len(plan)], mybir.dt.float32, tag="osb")
            for t, (r0, w) in enumerate(plan):
                b = C // w          # pieces per row
                f = w * HW          # floats per partition
                xt = xp.tile([128, max_w * HW], mybir.dt.float32, tag="xt")
                if len(rings) == len(plan):
                    eng = getattr(nc, rings[t])
                else:
                    eng = getattr(nc, rings[t % len(rings)])
                src = x[r0 : r0 + w // 2, :].rearrange("a (b f) -> (a b) f", b=b)
                eng.dma_start(out=xt[:, :f], in_=src)
                cs = accp.tile([128, max_w], mybir.dt.float32, tag="cs")
                if t in scalar_tiles:
                    # Tail tiles: the scalar engine does the per-channel
                    # hw-sums (Copy-activation with accum_out), freeing the
                    # vector engine, which is backlogged at the drain.
                    tr = trp.tile([128, HW], mybir.dt.float32, tag="tr")
                    for ch in range(w):
                        nc.scalar.activation(
                            tr,
                            xt[:, ch * HW : (ch + 1) * HW],
                            mybir.ActivationFunctionType.Copy,
                            accum_out=cs[:, ch : ch + 1],
                        )
                    d = w
                else:
                    d = w - min(folds[t], w)
                    # Emit reduces in <=32-channel chunks: 32-wide groups
                    # run at 1.066 ns/elem on DVE vs 1.26 at 64-wide.
                    for c0 in range(0, d, 32):
                        ce = min(c0 + 32, d)
                        nc.vector.reduce_sum(
                            cs[:, c0:ce],
                            xt[:, c0 * HW : ce * HW].rearrange(
                                "p (g w) -> p g w", w=HW
                            ),
                            axis=mybir.AxisListType.X,
                        )
                if d < w:
                    g = w - d
                    ft = fp.tile([128, max_fold * fw], mybir.dt.float32, tag="ft")
                    x3 = xt[:, d * HW : w * HW].rearrange(
                        "p (g w) -> p g w", w=HW
                    )
                    nc.gpsimd.tensor_add(
                        ft[:, : g * fw].rearrange("p (g w) -> p g w", w=fw),
                        x3[:, :, 0:fw],
                        x3[:, :, fw:HW],
                    )
                    nc.vector.reduce_sum(
                        cs[:, d:w],
                        ft[:, : g * fw].rearrange("p (g w) -> p g w", w=fw),
                        axis=mybir.AxisListType.X,
                    )
                if scalar_sq:
                    # Square + per-partition sum in one scalar op; keeps the
                    # vector engine free for the big reduces.
                    tr = accp.tile([128, max_w], mybir.dt.float32, tag="tr")
                    nc.scalar.activation(
                        tr[:, :w],
                        cs[:, :w],
                        mybir.ActivationFunctionType.Square,
                        accum_out=osb[:, t : t + 1],
                    )
                else:
                    sq = accp.tile([128, max_w], mybir.dt.float32, tag="sq")
                    nc.vector.tensor_mul(sq[:, :w], cs[:, :w], cs[:, :w])
                    nc.vector.reduce_sum(
                        osb[:, t : t + 1], sq[:, :w], axis=mybir.AxisListType.X
                    )
            nc.sync.dma_start(out=out[0:128, :], in_=osb)
    nc.finalize()
    return nc


def _get_nc():
    global _NC_CACHE
    if _NC_CACHE is None:
        _NC_CACHE = _build_nc_hwdge()
    return _NC_CACHE


def _norms_from_partials(partials):
    """partials: [128, N_TILES] per-core -> per-row sum of squares [228]."""
    nsq = np.zeros(ROWS_PER_CORE, dtype=np.float64)
    for t, (r0, w) in enumerate(TILE_PLAN):
        b = C // w
        ps = partials[:, t].astype(np.float64).reshape(w // 2, b).sum(axis=1)
        nsq[r0 : r0 + w // 2] += ps
    return np.sqrt(nsq) / float(HW)


def _zero_mask():
    mask = np.ones((M, M), dtype=np.float64)
    for i in range(NUM_SAMPLES):
        r = (1 + i) * NUM_NODES
        for c in range(1, M):
            if c % NUM_NODES != 0 and (c - 1) // NUM_NODES != i:
                mask[r, c] = 0.0
    return mask


def _pattern_mixer_np(mat, sigma, lin_w, lin_b, mixed_mat):
    mat = np.asarray(mat, np.float64)            # [5, 7, 7]
    sigma = np.asarray(sigma, np.float64)        # [4, 5, 1]
    lin_w = np.asarray(lin_w, np.float64)        # [4, 5]
    lin_b = np.asarray(lin_b, np.float64)        # [4]
    mixed_mat = np.asarray(mixed_mat, np.float64)  # [4, 57, 57]

    T2 = 2 * NUM_FRAME - 1  # 15
    dist = np.abs(np.arange(T2, dtype=np.float64) - (NUM_FRAME - 1))
    te = (1.0 / (np.sqrt(2.0 * np.pi) * sigma)) * np.exp(
        -(dist**2) / (2.0 * sigma**2)
    )  # [4, 5, 15]
    ce = 1.0 / (1.0 + np.exp(-te))
    mixed = (
        np.einsum("hbt,bnm,hb->hntm", ce, mat, lin_w)
        + lin_b[:, None, None, None]
    )
    mixed = np.maximum(mixed, 0.0).reshape(NUM_MIXED, NUM_NODES, T2 * NUM_NODES)
    blocks = [
        mixed[
            :,
            :,
            NUM_NODES * (NUM_SAMPLES - 1 - i) : NUM_NODES * (2 * NUM_SAMPLES - 1 - i),
        ]
        for i in range(NUM_SAMPLES)
    ]
    add_block = np.concatenate(blocks, axis=1)  # [4, 56, 56]
    mm = mixed_mat.copy()
    mm[:, 1:, 1:] += add_block
    mm *= _zero_mask()[None]
    deg = np.maximum(mm.sum(axis=2), 1.0) ** -0.5  # [4, 57]
    return (deg[:, :, None] * mm * deg[:, None, :]).astype(np.float32)


def kernel(mat, x, sigma, lin_w, lin_b, mixed_mat, alpha):
    global LAST_RESULT
    x = np.ascontiguousarray(np.asarray(x, dtype=np.float32))
    xs = x.reshape(ROWS_TOTAL, CW)
    in_maps = [
        {"x": xs[i * ROWS_PER_CORE : (i + 1) * ROWS_PER_CORE]} for i in range(N_CORES)
    ]
    nc = _get_nc()
    res = run_bass_kernel_spmd(nc, in_maps, core_ids=list(range(N_CORES)))
    LAST_RESULT = res
    norms = np.concatenate([_norms_from_partials(r["out"]) for r in res.results])
    nrm = norms.reshape(B, M).astype(np.float32)

    mp = _pattern_mixer_np(mat, sigma, lin_w, lin_b, mixed_mat)  # [4, 57, 57] f32
    alpha = np.asarray(alpha, np.float32).reshape(1, NUM_MIXED, 1, 1)
    out = mp[None] + alpha * nrm[:, None, None, :]
    return np.ascontiguousarray(out.astype(np.float32))



# revision 7
# speedup vs baseline: 1.2814x; 1.2814x over previous
"""Trainium2 Bass kernel for nn_Amplified_PatternMixer.

Computation:
  out[b, h, m1, m2] = mixed_pattern[h, m1, m2] + alpha[h] * nrm[b, m2]
where
  nrm[b, m] = || mean_{hw}(x[b*57+m, :, h, w]) ||_2   over channels
  mixed_pattern = tiny 57x57 graph-normalized pattern (from 5x7x7 params).

The memory-bound part (streaming x: [1824, 256, 14, 14] f32, ~366 MB) runs
on 8 NeuronCores, data-parallel over rows (228 rows/core).

Per-core layout: each row (256 ch x 196 hw) is split into 4 quarter-rows of
64 channels; the 912 quarter-rows tile as 7 x [128 part, 64ch*196] plus one
[128 part, 8ch*196] tail (rows 224..227 split 32-ways).  Every tile's HBM
source is one fully contiguous range fanned over all 128 partitions, so all
16 SDMA engines carry identical byte loads (57 * 50176 B each).

Loads are issued via HWDGE (nc.sync/nc.scalar) rather than SWDGE
(nc.gpsimd): SWDGE descriptor generation runs on the GpSimd Q7 cores, which
are locked out of the shared SBUF port while DVE reduce ops run - the
baseline trace showed per-engine DMA rate collapsing from 27.1 GB/s to
~21 GB/s whenever vector was active.  HWDGE descriptors are generated in
RTL and are immune.

Each tile: vector reduce over hw=196 -> per-channel sums cs[128, w];
scalar Square-activation with accum_out -> per-piece sum of squares
(one partial per partition per tile).  Host combines the 4 (or 32)
partials per row, sqrt, /196.  The tiny 57x57 pattern-mixer runs on host.
"""

import ml_dtypes
import numpy as np

import concourse.bacc as bacc
import concourse.mybir as mybir
import concourse.tile as tile
from concourse.bass_utils import run_bass_kernel_spmd

# Problem constants (hardcoded; kernel.py must be self-contained).
NUM_BASIC = 5
NUM_MIXED = 4
NUM_FRAME = 8
NUM_NODES = 7
NUM_SAMPLES = 8
M = 1 + NUM_NODES * NUM_FRAME  # 57

N_CORES = 8
B = 32
C = 256
HW = 196  # 14*14
ROWS_TOTAL = B * M          # 1824
ROWS_PER_CORE = ROWS_TOTAL // N_CORES  # 228
CW = C * HW                 # 50176 floats per row

# (row_start, w): tile covers rows row_start..row_start+w//2, as 128 pieces
# of w channels each (w/2 rows x 256/w pieces).  Per-partition run is
# w*196*4 bytes (<= 64KB keeps one descriptor per partition).
# Graded ramp + uniform 32ch body + tapered tail.  The ramp lets vector
# start ~1us after the first DMA lands; 32ch tiles hit the DVE reduce
# sweet spot (32-wide groups: 1.066 ns/elem vs 1.26 at 64-wide) and per
# tile vector work (6.7us) stays under DMA time (7.6us) so the pipeline
# is DMA-paced; deep buffering (bufs=4) keeps DMA issue from ever gating
# on vector; the taper keeps the post-DMA drain ~3us.  Best measured:
# 130.1us (quiet HBM); shared-HBM neighbor bursts add +10..25us to any
# config.  The 64ch-body + gpsimd-fold variant measured 133.0-133.4 in
# the same windows.
_WIDTHS = [4, 8, 16, 32] + [32] * 11 + [16, 16, 8, 4]
assert sum(_WIDTHS) == 2 * ROWS_PER_CORE
TILE_PLAN = []
_r = 0
for _w in _WIDTHS:
    TILE_PLAN.append((_r, _w))
    _r += _w // 2
N_TILES = len(TILE_PLAN)

# Per-tile GpSimd fold policy: number of channels whose hw-reduction is
# pre-halved (196 -> 98) by gpsimd.tensor_add before the vector reduce.
# Body 64-tiles fold half (raw reduce runs parallel to the fold); tail
# tiles fold (half for w=32, fully for w<=16) since gpsimd is idle there
# and vector is the drain path; the final tile stays raw (shortest
# latency chain).
def _fold_plan(widths):
    # With the 32ch body the vector engine keeps pace on raw reduces
    # (~95us total vs ~111us of DMA); gpsimd folding is not needed.
    return [0 for _ in widths]

_FOLDS = _fold_plan(_WIDTHS)

LAST_RESULT = None
_NC_CACHE = None


def _build_nc_hwdge(plan=TILE_PLAN, folds=_FOLDS, bufs=4,
                    rings=("sync", "scalar"), scalar_sq=True,
                    scalar_tiles=()):
    """HWDGE row-piece kernel.

    rings: cycle of engine names for the tile load DMAs (HWDGE engines:
    sync, scalar); alternating measured best.  Outstanding HWDGE DMAs
    time-share the SDMA engines (even on one queue), so completion sems
    fire ~(in-flight-behind bytes)/BW late -- another reason to keep
    tiles moderate and buffering deep.
    folds[t]: optional GpSimd pre-fold of the last folds[t] channels
    hw 196 -> 98 (tensor_add on its own SBUF door) to offload vector;
    unused in the default 32ch-body plan (vector keeps pace raw).
    bufs=4 ensures DMA issue (gated by consumer-done of the tile 4
    back) never waits on a vector hiccup.
    """
    nc = bacc.Bacc(None)
    x = nc.declare_dram_parameter(
        "x", [ROWS_PER_CORE, CW], mybir.dt.bfloat16, isOutput=False
    )
    out = nc.declare_dram_parameter(
        "out", [128, len(plan)], mybir.dt.float32, isOutput=True
    )
    max_w = max(w for _, w in plan)
    max_fold = max(folds) if folds else 0
    scalar_tiles = scalar_tiles or ()
    fw = HW // 2  # 98
    with tile.TileContext(nc) as tc:
        with (
            tc.tile_pool(name="xt_pool", bufs=bufs) as xp,
            tc.tile_pool(name="fold_pool", bufs=2) as fp,
            tc.tile_pool(name="acc_pool", bufs=2) as accp,
            tc.tile_pool(name="res_pool", bufs=1) as resp,
            tc.tile_pool(name="trash_pool", bufs=2) as trp,
        ):
            osb = resp.tile([128, len(plan)], mybir.dt.float32, tag="osb")
            for t, (r0, w) in enumerate(plan):
                b = C // w          # pieces per row
                f = w * HW          # floats per partition
                xt = xp.tile([128, max_w * HW], mybir.dt.bfloat16, tag="xt")
                if len(rings) == len(plan):
                    eng = getattr(nc, rings[t])
                else:
                    eng = getattr(nc, rings[t % len(rings)])
                src = x[r0 : r0 + w // 2, :].rearrange("a (b f) -> (a b) f", b=b)
                eng.dma_start(out=xt[:, :f], in_=src)
                # cs is bf16 so the DVE reduce qualifies for the 2x_1P packed
                # mode (all src+dst 2B); HW accumulates in fp32 internally and
                # only rounds the final per-channel sum, so precision is fine.
                cs = accp.tile([128, max_w], mybir.dt.bfloat16, tag="cs")
                if t in scalar_tiles:
                    # Tail tiles: the scalar engine does the per-channel
                    # hw-sums (Copy-activation with accum_out), freeing the
                    # vector engine, which is backlogged at the drain.
                    tr = trp.tile([128, HW], mybir.dt.float32, tag="tr")
                    for ch in range(w):
                        nc.scalar.activation(
                            tr,
                            xt[:, ch * HW : (ch + 1) * HW],
                            mybir.ActivationFunctionType.Copy,
                            accum_out=cs[:, ch : ch + 1],
                        )
                    d = w
                else:
                    d = w - min(folds[t], w)
                    # Emit reduces in <=32-channel chunks: 32-wide groups
                    # run at 1.066 ns/elem on DVE vs 1.26 at 64-wide.
                    for c0 in range(0, d, 32):
                        ce = min(c0 + 32, d)
                        with nc.allow_low_precision(
                            "bf16 reduce output; HW accumulates fp32 internally"
                        ):
                            nc.vector.reduce_sum(
                                cs[:, c0:ce],
                                xt[:, c0 * HW : ce * HW].rearrange(
                                    "p (g w) -> p g w", w=HW
                                ),
                                axis=mybir.AxisListType.X,
                            )
                if d < w:
                    g = w - d
                    ft = fp.tile([128, max_fold * fw], mybir.dt.float32, tag="ft")
                    x3 = xt[:, d * HW : w * HW].rearrange(
                        "p (g w) -> p g w", w=HW
                    )
                    nc.gpsimd.tensor_add(
                        ft[:, : g * fw].rearrange("p (g w) -> p g w", w=fw),
                        x3[:, :, 0:fw],
                        x3[:, :, fw:HW],
                    )
                    nc.vector.reduce_sum(
                        cs[:, d:w],
                        ft[:, : g * fw].rearrange("p (g w) -> p g w", w=fw),
                        axis=mybir.AxisListType.X,
                    )
                if scalar_sq:
                    # Square + per-partition sum in one scalar op; keeps the
                    # vector engine free for the big reduces.
                    tr = accp.tile([128, max_w], mybir.dt.float32, tag="tr")
                    nc.scalar.activation(
                        tr[:, :w],
                        cs[:, :w],
                        mybir.ActivationFunctionType.Square,
                        accum_out=osb[:, t : t + 1],
                    )
                else:
                    sq = accp.tile([128, max_w], mybir.dt.float32, tag="sq")
                    nc.vector.tensor_mul(sq[:, :w], cs[:, :w], cs[:, :w])
                    nc.vector.reduce_sum(
                        osb[:, t : t + 1], sq[:, :w], axis=mybir.AxisListType.X
                    )
            nc.sync.dma_start(out=out[0:128, :], in_=osb)
    nc.finalize()
    return nc


def _get_nc():
    global _NC_CACHE
    if _NC_CACHE is None:
        _NC_CACHE = _build_nc_hwdge()
    return _NC_CACHE


def _norms_from_partials(partials):
    """partials: [128, N_TILES] per-core -> per-row sum of squares [228]."""
    nsq = np.zeros(ROWS_PER_CORE, dtype=np.float64)
    for t, (r0, w) in enumerate(TILE_PLAN):
        b = C // w
        ps = partials[:, t].astype(np.float64).reshape(w // 2, b).sum(axis=1)
        nsq[r0 : r0 + w // 2] += ps
    return np.sqrt(nsq) / float(HW)


def _zero_mask():
    mask = np.ones((M, M), dtype=np.float64)
    for i in range(NUM_SAMPLES):
        r = (1 + i) * NUM_NODES
        for c in range(1, M):
            if c % NUM_NODES != 0 and (c - 1) // NUM_NODES != i:
                mask[r, c] = 0.0
    return mask


def _pattern_mixer_np(mat, sigma, lin_w, lin_b, mixed_mat):
    mat = np.asarray(mat, np.float64)            # [5, 7, 7]
    sigma = np.asarray(sigma, np.float64)        # [4, 5, 1]
    lin_w = np.asarray(lin_w, np.float64)        # [4, 5]
    lin_b = np.asarray(lin_b, np.float64)        # [4]
    mixed_mat = np.asarray(mixed_mat, np.float64)  # [4, 57, 57]

    T2 = 2 * NUM_FRAME - 1  # 15
    dist = np.abs(np.arange(T2, dtype=np.float64) - (NUM_FRAME - 1))
    te = (1.0 / (np.sqrt(2.0 * np.pi) * sigma)) * np.exp(
        -(dist**2) / (2.0 * sigma**2)
    )  # [4, 5, 15]
    ce = 1.0 / (1.0 + np.exp(-te))
    mixed = (
        np.einsum("hbt,bnm,hb->hntm", ce, mat, lin_w)
        + lin_b[:, None, None, None]
    )
    mixed = np.maximum(mixed, 0.0).reshape(NUM_MIXED, NUM_NODES, T2 * NUM_NODES)
    blocks = [
        mixed[
            :,
            :,
            NUM_NODES * (NUM_SAMPLES - 1 - i) : NUM_NODES * (2 * NUM_SAMPLES - 1 - i),
        ]
        for i in range(NUM_SAMPLES)
    ]
    add_block = np.concatenate(blocks, axis=1)  # [4, 56, 56]
    mm = mixed_mat.copy()
    mm[:, 1:, 1:] += add_block
    mm *= _zero_mask()[None]
    deg = np.maximum(mm.sum(axis=2), 1.0) ** -0.5  # [4, 57]
    return (deg[:, :, None] * mm * deg[:, None, :]).astype(np.float32)


def kernel(mat, x, sigma, lin_w, lin_b, mixed_mat, alpha):
    global LAST_RESULT
    x = np.asarray(x, dtype=np.float32)
    xs = np.ascontiguousarray(x.reshape(ROWS_TOTAL, CW)).astype(ml_dtypes.bfloat16)
    in_maps = [
        {"x": xs[i * ROWS_PER_CORE : (i + 1) * ROWS_PER_CORE]} for i in range(N_CORES)
    ]
    nc = _get_nc()
    res = run_bass_kernel_spmd(nc, in_maps, core_ids=list(range(N_CORES)))
    LAST_RESULT = res
    norms = np.concatenate([_norms_from_partials(r["out"]) for r in res.results])
    nrm = norms.reshape(B, M).astype(np.float32)

    mp = _pattern_mixer_np(mat, sigma, lin_w, lin_b, mixed_mat)  # [4, 57, 57] f32
    alpha = np.asarray(alpha, np.float32).reshape(1, NUM_MIXED, 1, 1)
    out = mp[None] + alpha * nrm[:, None, None, :]
    return np.ascontiguousarray(out.astype(np.float32))



# revision 10
# speedup vs baseline: 1.4469x; 1.1292x over previous
"""Trainium2 Bass kernel for nn_Amplified_PatternMixer.

Computation:
  out[b, h, m1, m2] = mixed_pattern[h, m1, m2] + alpha[h] * nrm[b, m2]
where
  nrm[b, m] = || mean_{hw}(x[b*57+m, :, h, w]) ||_2   over channels
  mixed_pattern = tiny 57x57 graph-normalized pattern (from 5x7x7 params).

The memory-bound part (streaming x: [1824, 256, 14, 14]) runs on 8
NeuronCores, data-parallel over rows (228 rows/core).

Optimization history / HW facts (measured on this trn2):
  * f32 stream was DMA-bound at ~130-156us (45.8 MB/core, ~23-27 B/ns
    per SDMA engine x16).
  * Upload dtype is ours to choose: host casts x to bf16 (RNE).  The
    pooled-mean + channel-norm averages away the 0.4% per-element
    quantization noise (measured end-to-end rel err ~2e-4 vs the 2e-2
    gate).  Halves DMA bytes -> ~62us floor.
  * DVE TENSOR_REDUCE only has a 1x uop (1 elem/cycle/lane at ANY
    dtype; bf16 run measured 1.064 ns/elem, same as f32), so a plain
    reduce is a 93us floor -> restructure as a TT tree: bf16
    tensor_tensor DOES hit the 2x_1P packed mode (2 outs = 4 ins per
    cycle, measured 0.55 cyc/out), but only while every src/dst is
    2-byte, step +-1 and 4-byte aligned.  196 = 4*49 breaks alignment
    at the second level, so the host pads each channel 196 -> 200
    zeros (+2% DMA): L1 200->100 and L2 100->50 both stay packed, the
    final 50-wide reduce runs at 1x.  Measured 4626 ns per 32-channel
    tile = 145 ns/channel-column -> ~66us DVE for the full core load.
  * GpSimd tensor_add (bf16) measured 175 ns/channel-column for the L1
    halve -- ~3x slower than DVE but it is an otherwise-idle engine, so
    it folds a fraction of each tile's channels to pull DVE below the
    DMA floor.
  * ScalarE ACTIVATE+READ_ACCUMULATOR costs ~740 ns per accumulate
    instruction pair regardless of size -> only used for the tiny
    per-tile Square(cs) -> sum-of-squares partials.
  * fp8 would halve DMA again but no engine reduces fp8 faster than
    1 elem/cycle (DVE fp8 TT measured 1.04 cyc/out, no packing; PE
    LDW+MM measured 104+165 ns per 128x128 block = way under the
    needed rate), so fp8 upload is compute-dead.  bf16 is optimal.

Loads are issued via HWDGE (nc.sync/nc.scalar); SWDGE (gpsimd) was
measured to collapse DMA rate while DVE runs (shared SBUF port).
"""

import ml_dtypes
import numpy as np

import concourse.bacc as bacc
import concourse.mybir as mybir
import concourse.tile as tile
from concourse.bass_utils import run_bass_kernel_spmd

# Problem constants (hardcoded; kernel.py must be self-contained).
NUM_BASIC = 5
NUM_MIXED = 4
NUM_FRAME = 8
NUM_NODES = 7
NUM_SAMPLES = 8
M = 1 + NUM_NODES * NUM_FRAME  # 57

N_CORES = 8
B = 32
C = 256
HW = 196   # 14*14 (true)
WP = 200   # host-padded channel width (4 zeros) to keep the TT tree
           # 4B-aligned at every level: 200 -> 100 -> 50
ROWS_TOTAL = B * M          # 1824
ROWS_PER_CORE = ROWS_TOTAL // N_CORES  # 228
CWP = C * WP                # 51200 bf16 per row (padded)

# (row_start, w, pool_ch): tile covers rows row_start..row_start+w//2 as
# 128 pieces of w channels (w/2 rows x 256/w pieces); per-partition DMA
# run = w*200*2 B, one descriptor per partition.  pool_ch leading
# channels of each tile are L1-folded on GpSimd instead of DVE.
# Ramp lets DVE start early; 64-wide body maximizes descriptor size
# (25.6 KB) for DMA efficiency; taper keeps the drain short.
_WIDTHS = [4, 8, 16, 32, 64, 64, 64, 64, 64, 32, 16, 16, 8, 4]
assert sum(_WIDTHS) == 2 * ROWS_PER_CORE
assert all(C % w == 0 for w in _WIDTHS)


def _pool_share(w):
    # GpSimd folds ~3/8 of big-tile channels (must stay under DMA
    # cadence; POOL is ~3x slower than DVE per channel).
    return (w * 3 // 8) if w >= 32 else 0


TILE_PLAN = []
_r = 0
for _w in _WIDTHS:
    TILE_PLAN.append((_r, _w, _pool_share(_w)))
    _r += _w // 2
N_TILES = len(TILE_PLAN)

LAST_RESULT = None
_NC_CACHE = None


def _build_nc(plan=TILE_PLAN, bufs=4, rings=("sync", "scalar")):
    nc = bacc.Bacc(None)
    x = nc.declare_dram_parameter(
        "x", [ROWS_PER_CORE, CWP], mybir.dt.bfloat16, isOutput=False
    )
    out = nc.declare_dram_parameter(
        "out", [128, len(plan)], mybir.dt.float32, isOutput=True
    )
    max_w = max(w for _, w, _ in plan)
    with tile.TileContext(nc) as tc:
        with (
            tc.tile_pool(name="xt_pool", bufs=bufs) as xp,
            tc.tile_pool(name="ft_pool", bufs=2) as fp,
            tc.tile_pool(name="gt_pool", bufs=2) as gp,
            tc.tile_pool(name="acc_pool", bufs=2) as accp,
            tc.tile_pool(name="res_pool", bufs=1) as resp,
        ):
            osb = resp.tile([128, len(plan)], mybir.dt.float32, tag="osb")
            for t, (r0, w, pc) in enumerate(plan):
                b = C // w          # pieces per row
                f = w * WP          # bf16 elems per partition
                xt = xp.tile([128, max_w * WP], mybir.dt.bfloat16, tag="xt")
                eng = getattr(nc, rings[t % len(rings)])
                src = x[r0 : r0 + w // 2, :].rearrange("a (b f) -> (a b) f", b=b)
                eng.dma_start(out=xt[:, :f], in_=src)

                x3 = xt[:, :f].rearrange("p (g v) -> p g v", v=WP)
                ft = fp.tile([128, max_w * 100], mybir.dt.bfloat16, tag="ft")
                f3 = ft[:, : w * 100].rearrange("p (g v) -> p g v", v=100)
                # L1: 200 -> 100.  GpSimd folds the first pc channels,
                # DVE the rest (both 2-byte packed / native paths).
                if pc:
                    nc.gpsimd.tensor_add(
                        f3[:, 0:pc, :], x3[:, 0:pc, 0:100], x3[:, 0:pc, 100:200]
                    )
                if pc < w:
                    nc.vector.tensor_add(
                        f3[:, pc:w, :], x3[:, pc:w, 0:100], x3[:, pc:w, 100:200]
                    )
                # L2: 100 -> 50 (DVE, packed)
                gt = gp.tile([128, max_w * 50], mybir.dt.bfloat16, tag="gt")
                g3 = gt[:, : w * 50].rearrange("p (g v) -> p g v", v=50)
                nc.vector.tensor_add(g3, f3[:, :, 0:50], f3[:, :, 50:100])
                # final 50-wide reduce (1x) -> per-channel sums
                cs = accp.tile([128, max_w], mybir.dt.bfloat16, tag="cs")
                with nc.allow_low_precision("bf16 sums; fp32 internal accum"):
                    nc.vector.reduce_sum(cs[:, :w], g3, axis=mybir.AxisListType.X)
                # Square + per-partition sum on the (idle) scalar engine.
                tr = accp.tile([128, max_w], mybir.dt.float32, tag="tr")
                nc.scalar.activation(
                    tr[:, :w],
                    cs[:, :w],
                    mybir.ActivationFunctionType.Square,
                    accum_out=osb[:, t : t + 1],
                )
            nc.sync.dma_start(out=out[0:128, :], in_=osb)
    nc.finalize()
    return nc


def _get_nc():
    global _NC_CACHE
    if _NC_CACHE is None:
        _NC_CACHE = _build_nc()
    return _NC_CACHE


def _norms_from_partials(partials):
    """partials: [128, N_TILES] per-core -> per-row norms [228]."""
    nsq = np.zeros(ROWS_PER_CORE, dtype=np.float64)
    for t, (r0, w, _) in enumerate(TILE_PLAN):
        b = C // w
        ps = partials[:, t].astype(np.float64).reshape(w // 2, b).sum(axis=1)
        nsq[r0 : r0 + w // 2] += ps
    return np.sqrt(nsq) / float(HW)


def _zero_mask():
    mask = np.ones((M, M), dtype=np.float64)
    for i in range(NUM_SAMPLES):
        r = (1 + i) * NUM_NODES
        for c in range(1, M):
            if c % NUM_NODES != 0 and (c - 1) // NUM_NODES != i:
                mask[r, c] = 0.0
    return mask


def _pattern_mixer_np(mat, sigma, lin_w, lin_b, mixed_mat):
    mat = np.asarray(mat, np.float64)            # [5, 7, 7]
    sigma = np.asarray(sigma, np.float64)        # [4, 5, 1]
    lin_w = np.asarray(lin_w, np.float64)        # [4, 5]
    lin_b = np.asarray(lin_b, np.float64)        # [4]
    mixed_mat = np.asarray(mixed_mat, np.float64)  # [4, 57, 57]

    T2 = 2 * NUM_FRAME - 1  # 15
    dist = np.abs(np.arange(T2, dtype=np.float64) - (NUM_FRAME - 1))
    te = (1.0 / (np.sqrt(2.0 * np.pi) * sigma)) * np.exp(
        -(dist**2) / (2.0 * sigma**2)
    )  # [4, 5, 15]
    ce = 1.0 / (1.0 + np.exp(-te))
    mixed = (
        np.einsum("hbt,bnm,hb->hntm", ce, mat, lin_w)
        + lin_b[:, None, None, None]
    )
    mixed = np.maximum(mixed, 0.0).reshape(NUM_MIXED, NUM_NODES, T2 * NUM_NODES)
    blocks = [
        mixed[
            :,
            :,
            NUM_NODES * (NUM_SAMPLES - 1 - i) : NUM_NODES * (2 * NUM_SAMPLES - 1 - i),
        ]
        for i in range(NUM_SAMPLES)
    ]
    add_block = np.concatenate(blocks, axis=1)  # [4, 56, 56]
    mm = mixed_mat.copy()
    mm[:, 1:, 1:] += add_block
    mm *= _zero_mask()[None]
    deg = np.maximum(mm.sum(axis=2), 1.0) ** -0.5  # [4, 57]
    return (deg[:, :, None] * mm * deg[:, None, :]).astype(np.float32)


def kernel(mat, x, sigma, lin_w, lin_b, mixed_mat, alpha):
    global LAST_RESULT
    xf = np.asarray(x, dtype=np.float32).reshape(ROWS_TOTAL, C, HW)
    xs = np.zeros((ROWS_TOTAL, C, WP), dtype=ml_dtypes.bfloat16)
    xs[:, :, :HW] = xf.astype(ml_dtypes.bfloat16)
    xs = xs.reshape(ROWS_TOTAL, CWP)
    in_maps = [
        {"x": xs[i * ROWS_PER_CORE : (i + 1) * ROWS_PER_CORE]} for i in range(N_CORES)
    ]
    nc = _get_nc()
    res = run_bass_kernel_spmd(nc, in_maps, core_ids=list(range(N_CORES)))
    LAST_RESULT = res
    norms = np.concatenate([_norms_from_partials(r["out"]) for r in res.results])
    nrm = norms.reshape(B, M).astype(np.float32)

    mp = _pattern_mixer_np(mat, sigma, lin_w, lin_b, mixed_mat)  # [4, 57, 57] f32
    alpha = np.asarray(alpha, np.float32).reshape(1, NUM_MIXED, 1, 1)
    out = mp[None] + alpha * nrm[:, None, None, :]
    return np.ascontiguousarray(out.astype(np.float32))


# revision 13
# speedup vs baseline: 1.4665x; 1.0136x over previous
"""Trainium2 Bass kernel for nn_Amplified_PatternMixer.

Computation:
  out[b, h, m1, m2] = mixed_pattern[h, m1, m2] + alpha[h] * nrm[b, m2]
where
  nrm[b, m] = || mean_{hw}(x[b*57+m, :, h, w]) ||_2   over channels
  mixed_pattern = tiny 57x57 graph-normalized pattern (from 5x7x7 params).

The memory-bound part (streaming x: [1824, 256, 14, 14]) runs on 8
NeuronCores, data-parallel over rows (228 rows/core).

Optimization history / HW facts (measured on this trn2):
  * f32 stream was DMA-bound at ~130-156us (45.8 MB/core, ~23-27 B/ns
    per SDMA engine x16).
  * Upload dtype is ours to choose: host casts x to bf16 (RNE).  The
    pooled-mean + channel-norm averages away the 0.4% per-element
    quantization noise (measured end-to-end rel err ~2e-4 vs the 2e-2
    gate).  Halves DMA bytes -> ~62us floor.
  * DVE TENSOR_REDUCE only has a 1x uop (1 elem/cycle/lane at ANY
    dtype; bf16 run measured 1.064 ns/elem, same as f32), so a plain
    reduce is a 93us floor -> restructure as a TT tree: bf16
    tensor_tensor DOES hit the 2x_1P packed mode (2 outs = 4 ins per
    cycle, measured 0.55 cyc/out), but only while every src/dst is
    2-byte, step +-1 and 4-byte aligned.  196 = 4*49 breaks alignment
    at the second level, so the host pads each channel 196 -> 200
    zeros (+2% DMA): L1 200->100 and L2 100->50 both stay packed, the
    final 50-wide reduce runs at 1x.  Measured 4626 ns per 32-channel
    tile = 145 ns/channel-column -> ~66us DVE for the full core load.
  * GpSimd tensor_add (bf16) measured 175 ns/channel-column for the L1
    halve -- ~3x slower than DVE but it is an otherwise-idle engine, so
    it folds a fraction of each tile's channels to pull DVE below the
    DMA floor.
  * ScalarE ACTIVATE+READ_ACCUMULATOR costs ~740 ns per accumulate
    instruction pair regardless of size -> only used for the tiny
    per-tile Square(cs) -> sum-of-squares partials.
  * fp8 would halve DMA again but no engine reduces fp8 faster than
    1 elem/cycle (DVE fp8 TT measured 1.04 cyc/out, no packing; PE
    LDW+MM measured 104+165 ns per 128x128 block = way under the
    needed rate), so fp8 upload is compute-dead.  bf16 is optimal.

Loads are issued via HWDGE (nc.sync/nc.scalar); SWDGE (gpsimd) was
measured to collapse DMA rate while DVE runs (shared SBUF port).
"""

import ml_dtypes
import numpy as np

import concourse.bacc as bacc
import concourse.mybir as mybir
import concourse.tile as tile
from concourse.bass_utils import run_bass_kernel_spmd

# Problem constants (hardcoded; kernel.py must be self-contained).
NUM_BASIC = 5
NUM_MIXED = 4
NUM_FRAME = 8
NUM_NODES = 7
NUM_SAMPLES = 8
M = 1 + NUM_NODES * NUM_FRAME  # 57

N_CORES = 8
B = 32
C = 256
HW = 196   # 14*14 (true)
WP = 200   # host-padded channel width (4 zeros) to keep the TT tree
           # 4B-aligned at every level: 200 -> 100 -> 50
ROWS_TOTAL = B * M          # 1824
ROWS_PER_CORE = ROWS_TOTAL // N_CORES  # 228
CWP = C * WP                # 51200 bf16 per row (padded)

# (row_start, w, pool_ch): tile covers rows row_start..row_start+w//2 as
# 128 pieces of w channels (w/2 rows x 256/w pieces); per-partition DMA
# run = w*200*2 B, one descriptor per partition.  pool_ch leading
# channels of each tile are L1-folded on GpSimd instead of DVE.
# Ramp lets DVE start early; 64-wide body maximizes descriptor size
# (25.6 KB) for DMA efficiency; taper keeps the drain short.
_WIDTHS = [4, 8, 16, 32, 64, 64, 64, 64, 64, 32, 16, 16, 8, 4]
assert sum(_WIDTHS) == 2 * ROWS_PER_CORE
assert all(C % w == 0 for w in _WIDTHS)


def _pool_share(w):
    # GpSimd folds half of big-tile channels (must stay under DMA
    # cadence; POOL is ~3x slower than DVE per channel).
    return (w // 2) if w >= 32 else 0


TILE_PLAN = []
_r = 0
for _w in _WIDTHS:
    TILE_PLAN.append((_r, _w, _pool_share(_w)))
    _r += _w // 2
N_TILES = len(TILE_PLAN)

LAST_RESULT = None
_NC_CACHE = None


def _build_nc(plan=TILE_PLAN, bufs=4, rings=("sync", "scalar")):
    nc = bacc.Bacc(None)
    x = nc.declare_dram_parameter(
        "x", [ROWS_PER_CORE, CWP], mybir.dt.bfloat16, isOutput=False
    )
    out = nc.declare_dram_parameter(
        "out", [128, len(plan)], mybir.dt.float32, isOutput=True
    )
    max_w = max(w for _, w, _ in plan)
    with tile.TileContext(nc) as tc:
        with (
            tc.tile_pool(name="xt_pool", bufs=bufs) as xp,
            tc.tile_pool(name="ft_pool", bufs=3) as fp,
            tc.tile_pool(name="gt_pool", bufs=2) as gp,
            tc.tile_pool(name="acc_pool", bufs=2) as accp,
            tc.tile_pool(name="res_pool", bufs=1) as resp,
        ):
            osb = resp.tile([128, len(plan)], mybir.dt.float32, tag="osb")

            def finish(t, w, f3):
                # L2: 100 -> 50 (DVE, packed), final 50-wide reduce (1x),
                # then Square + per-partition sum on the scalar engine.
                gt = gp.tile([128, max_w * 50], mybir.dt.bfloat16, tag="gt")
                g3 = gt[:, : w * 50].rearrange("p (g v) -> p g v", v=50)
                nc.vector.tensor_add(g3, f3[:, :, 0:50], f3[:, :, 50:100])
                cs = accp.tile([128, max_w], mybir.dt.bfloat16, tag="cs")
                with nc.allow_low_precision("bf16 sums; fp32 internal accum"):
                    nc.vector.reduce_sum(cs[:, :w], g3, axis=mybir.AxisListType.X)
                tr = accp.tile([128, max_w], mybir.dt.float32, tag="tr")
                nc.scalar.activation(
                    tr[:, :w],
                    cs[:, :w],
                    mybir.ActivationFunctionType.Square,
                    accum_out=osb[:, t : t + 1],
                )

            # Software-pipelined by one tile: DVE runs L2/reduce for tile
            # t-1 while GpSimd's (slower) L1 fold for tile t is still in
            # flight -- otherwise DVE's in-order queue stalls on the POOL
            # semaphore (~2.5us/tile measured).
            pending = None
            for t, (r0, w, pc) in enumerate(plan):
                b = C // w          # pieces per row
                f = w * WP          # bf16 elems per partition
                xt = xp.tile([128, max_w * WP], mybir.dt.bfloat16, tag="xt")
                eng = getattr(nc, rings[t % len(rings)])
                src = x[r0 : r0 + w // 2, :].rearrange("a (b f) -> (a b) f", b=b)
                eng.dma_start(out=xt[:, :f], in_=src)

                x3 = xt[:, :f].rearrange("p (g v) -> p g v", v=WP)
                ft = fp.tile([128, max_w * 100], mybir.dt.bfloat16, tag="ft")
                f3 = ft[:, : w * 100].rearrange("p (g v) -> p g v", v=100)
                # L1: 200 -> 100.  GpSimd folds the first pc channels,
                # DVE the rest (both on their fast 2-byte paths).
                if pc:
                    nc.gpsimd.tensor_add(
                        f3[:, 0:pc, :], x3[:, 0:pc, 0:100], x3[:, 0:pc, 100:200]
                    )
                if pc < w:
                    nc.vector.tensor_add(
                        f3[:, pc:w, :], x3[:, pc:w, 0:100], x3[:, pc:w, 100:200]
                    )
                if pending is not None:
                    finish(*pending)
                pending = (t, w, f3)
            finish(*pending)
            nc.sync.dma_start(out=out[0:128, :], in_=osb)
    nc.finalize()
    return nc


def _get_nc():
    global _NC_CACHE
    if _NC_CACHE is None:
        _NC_CACHE = _build_nc()
    return _NC_CACHE


def _norms_from_partials(partials):
    """partials: [128, N_TILES] per-core -> per-row norms [228]."""
    nsq = np.zeros(ROWS_PER_CORE, dtype=np.float64)
    for t, (r0, w, _) in enumerate(TILE_PLAN):
        b = C // w
        ps = partials[:, t].astype(np.float64).reshape(w // 2, b).sum(axis=1)
        nsq[r0 : r0 + w // 2] += ps
    return np.sqrt(nsq) / float(HW)


def _zero_mask():
    mask = np.ones((M, M), dtype=np.float64)
    for i in range(NUM_SAMPLES):
        r = (1 + i) * NUM_NODES
        for c in range(1, M):
            if c % NUM_NODES != 0 and (c - 1) // NUM_NODES != i:
                mask[r, c] = 0.0
    return mask


def _pattern_mixer_np(mat, sigma, lin_w, lin_b, mixed_mat):
    mat = np.asarray(mat, np.float64)            # [5, 7, 7]
    sigma = np.asarray(sigma, np.float64)        # [4, 5, 1]
    lin_w = np.asarray(lin_w, np.float64)        # [4, 5]
    lin_b = np.asarray(lin_b, np.float64)        # [4]
    mixed_mat = np.asarray(mixed_mat, np.float64)  # [4, 57, 57]

    T2 = 2 * NUM_FRAME - 1  # 15
    dist = np.abs(np.arange(T2, dtype=np.float64) - (NUM_FRAME - 1))
    te = (1.0 / (np.sqrt(2.0 * np.pi) * sigma)) * np.exp(
        -(dist**2) / (2.0 * sigma**2)
    )  # [4, 5, 15]
    ce = 1.0 / (1.0 + np.exp(-te))
    mixed = (
        np.einsum("hbt,bnm,hb->hntm", ce, mat, lin_w)
        + lin_b[:, None, None, None]
    )
    mixed = np.maximum(mixed, 0.0).reshape(NUM_MIXED, NUM_NODES, T2 * NUM_NODES)
    blocks = [
        mixed[
            :,
            :,
            NUM_NODES * (NUM_SAMPLES - 1 - i) : NUM_NODES * (2 * NUM_SAMPLES - 1 - i),
        ]
        for i in range(NUM_SAMPLES)
    ]
    add_block = np.concatenate(blocks, axis=1)  # [4, 56, 56]
    mm = mixed_mat.copy()
    mm[:, 1:, 1:] += add_block
    mm *= _zero_mask()[None]
    deg = np.maximum(mm.sum(axis=2), 1.0) ** -0.5  # [4, 57]
    return (deg[:, :, None] * mm * deg[:, None, :]).astype(np.float32)


def kernel(mat, x, sigma, lin_w, lin_b, mixed_mat, alpha):
    global LAST_RESULT
    xf = np.asarray(x, dtype=np.float32).reshape(ROWS_TOTAL, C, HW)
    xs = np.zeros((ROWS_TOTAL, C, WP), dtype=ml_dtypes.bfloat16)
    xs[:, :, :HW] = xf.astype(ml_dtypes.bfloat16)
    xs = xs.reshape(ROWS_TOTAL, CWP)
    in_maps = [
        {"x": xs[i * ROWS_PER_CORE : (i + 1) * ROWS_PER_CORE]} for i in range(N_CORES)
    ]
    nc = _get_nc()
    res = run_bass_kernel_spmd(nc, in_maps, core_ids=list(range(N_CORES)))
    LAST_RESULT = res
    norms = np.concatenate([_norms_from_partials(r["out"]) for r in res.results])
    nrm = norms.reshape(B, M).astype(np.float32)

    mp = _pattern_mixer_np(mat, sigma, lin_w, lin_b, mixed_mat)  # [4, 57, 57] f32
    alpha = np.asarray(alpha, np.float32).reshape(1, NUM_MIXED, 1, 1)
    out = mp[None] + alpha * nrm[:, None, None, :]
    return np.ascontiguousarray(out.astype(np.float32))


# revision 16
# speedup vs baseline: 1.5854x; 1.0810x over previous
"""Trainium2 Bass kernel for nn_Amplified_PatternMixer.

Computation:
  out[b, h, m1, m2] = mixed_pattern[h, m1, m2] + alpha[h] * nrm[b, m2]
where
  nrm[b, m] = || mean_{hw}(x[b*57+m, :, h, w]) ||_2   over channels
  mixed_pattern = tiny 57x57 graph-normalized pattern (from 5x7x7 params).

The memory-bound part (streaming x: [1824, 256, 14, 14]) runs on 8
NeuronCores, data-parallel over rows (228 rows/core).

Optimization history / HW facts (measured on this trn2):
  * f32 stream was DMA-bound at ~130-156us (45.8 MB/core, ~23-27 B/ns
    per SDMA engine x16).
  * Upload dtype is ours to choose: host casts x to bf16 (RNE).  The
    pooled-mean + channel-norm averages away the 0.4% per-element
    quantization noise (measured end-to-end rel err ~2e-4 vs the 2e-2
    gate).  Halves DMA bytes -> ~62us floor.
  * DVE TENSOR_REDUCE only has a 1x uop (1 elem/cycle/lane at ANY
    dtype; bf16 run measured 1.064 ns/elem, same as f32), so a plain
    reduce is a 93us floor -> restructure as a TT tree: bf16
    tensor_tensor DOES hit the 2x_1P packed mode (2 outs = 4 ins per
    cycle, measured 0.55 cyc/out), but only while every src/dst is
    2-byte, step +-1 and 4-byte aligned.  196 = 4*49 breaks alignment
    at the second level, so the host pads each channel 196 -> 200
    zeros (+2% DMA): L1 200->100 and L2 100->50 both stay packed, the
    final 50-wide reduce runs at 1x.  Measured 4626 ns per 32-channel
    tile = 145 ns/channel-column -> ~66us DVE for the full core load.
  * GpSimd tensor_add (bf16) measured 175 ns/channel-column for the L1
    halve -- ~3x slower than DVE but it is an otherwise-idle engine, so
    it folds a fraction of each tile's channels to pull DVE below the
    DMA floor.
  * ScalarE ACTIVATE+READ_ACCUMULATOR costs ~740 ns per accumulate
    instruction pair regardless of size -> only used for the tiny
    per-tile Square(cs) -> sum-of-squares partials.
  * fp8 would halve DMA again but no engine reduces fp8 faster than
    1 elem/cycle (DVE fp8 TT measured 1.04 cyc/out, no packing; PE
    LDW+MM measured 104+165 ns per 128x128 block = way under the
    needed rate), so fp8 upload is compute-dead.  bf16 is optimal.

Loads are issued via HWDGE (nc.sync/nc.scalar); SWDGE (gpsimd) was
measured to collapse DMA rate while DVE runs (shared SBUF port).
"""

import ml_dtypes
import numpy as np

import concourse.bacc as bacc
import concourse.mybir as mybir
import concourse.tile as tile
from concourse.bass_utils import run_bass_kernel_spmd

# Problem constants (hardcoded; kernel.py must be self-contained).
NUM_BASIC = 5
NUM_MIXED = 4
NUM_FRAME = 8
NUM_NODES = 7
NUM_SAMPLES = 8
M = 1 + NUM_NODES * NUM_FRAME  # 57

N_CORES = 8
B = 32
C = 256
HW = 196   # 14*14 (true)
WP = 200   # host-padded channel width (4 zeros) to keep the TT tree
           # 4B-aligned at every level: 200 -> 100 -> 50
ROWS_TOTAL = B * M          # 1824
ROWS_PER_CORE = ROWS_TOTAL // N_CORES  # 228
CWP = C * WP                # 51200 bf16 per row (padded)

# (row_start, w, pool_ch): tile covers rows row_start..row_start+w//2 as
# 128 pieces of w channels (w/2 rows x 256/w pieces); per-partition DMA
# run = w*200*2 B, one descriptor per partition.  pool_ch leading
# channels of each tile are L1-folded on GpSimd instead of DVE.
# Ramp lets DVE start early; 64-wide body maximizes descriptor size
# (25.6 KB) for DMA efficiency; taper keeps the drain short.
_WIDTHS = [4, 8, 16, 32, 64, 64, 64, 64, 64, 32, 16, 16, 8, 4]
assert sum(_WIDTHS) == 2 * ROWS_PER_CORE
assert all(C % w == 0 for w in _WIDTHS)


def _pool_share(w):
    # GpSimd folds a slice of big-tile channels.  POOL's tensor_add
    # contends with DVE's 2x-packed TENSOR_TENSOR for the shared SBUF
    # port (measured 2-4x TT slowdown when overlapped) but leaves the
    # 1x TENSOR_REDUCE untouched, so the share is sized to fit inside
    # the previous tile's reduce window (~3.6us for w=64): 20 channels
    # x 175ns = 3.5us.
    return (5 * w // 16) if w >= 32 else 0


TILE_PLAN = []
_r = 0
for _w in _WIDTHS:
    TILE_PLAN.append((_r, _w, _pool_share(_w)))
    _r += _w // 2
N_TILES = len(TILE_PLAN)

LAST_RESULT = None
_NC_CACHE = None


def _build_nc(plan=TILE_PLAN, bufs=4, rings=("sync", "scalar")):
    nc = bacc.Bacc(None)
    x = nc.declare_dram_parameter(
        "x", [ROWS_PER_CORE, CWP], mybir.dt.bfloat16, isOutput=False
    )
    out = nc.declare_dram_parameter(
        "out", [128, len(plan)], mybir.dt.float32, isOutput=True
    )
    max_w = max(w for _, w, _ in plan)
    with tile.TileContext(nc) as tc:
        with (
            tc.tile_pool(name="xt_pool", bufs=bufs) as xp,
            tc.tile_pool(name="ft_pool", bufs=3) as fp,
            tc.tile_pool(name="gt_pool", bufs=2) as gp,
            tc.tile_pool(name="acc_pool", bufs=2) as accp,
            tc.tile_pool(name="res_pool", bufs=1) as resp,
        ):
            osb = resp.tile([128, len(plan)], mybir.dt.float32, tag="osb")

            def finish(t, w, g3):
                # Final 50-wide reduce (1x) -> per-channel sums, then
                # Square + per-partition sum on the scalar engine.
                cs = accp.tile([128, max_w], mybir.dt.bfloat16, tag="cs")
                with nc.allow_low_precision("bf16 sums; fp32 internal accum"):
                    nc.vector.reduce_sum(cs[:, :w], g3, axis=mybir.AxisListType.X)
                tr = accp.tile([128, max_w], mybir.dt.float32, tag="tr")
                nc.scalar.activation(
                    tr[:, :w],
                    cs[:, :w],
                    mybir.ActivationFunctionType.Square,
                    accum_out=osb[:, t : t + 1],
                )

            # Software-pipelined by one tile, with the DVE queue ordered
            # [reduce(t-1), L1b(t), L2(t)]: GpSimd's L1a(t) fold starts
            # when DMA(t) lands, which is when DVE starts the (port-
            # light, contention-immune) reduce of t-1 -- so POOL never
            # overlaps a packed TT, and DVE never stalls on the POOL
            # semaphore (L2(t) issues after both L1 halves).
            pending = None
            for t, (r0, w, pc) in enumerate(plan):
                b = C // w          # pieces per row
                f = w * WP          # bf16 elems per partition
                xt = xp.tile([128, max_w * WP], mybir.dt.bfloat16, tag="xt")
                eng = getattr(nc, rings[t % len(rings)])
                src = x[r0 : r0 + w // 2, :].rearrange("a (b f) -> (a b) f", b=b)
                eng.dma_start(out=xt[:, :f], in_=src)

                x3 = xt[:, :f].rearrange("p (g v) -> p g v", v=WP)
                ft = fp.tile([128, max_w * 100], mybir.dt.bfloat16, tag="ft")
                f3 = ft[:, : w * 100].rearrange("p (g v) -> p g v", v=100)
                # L1a: GpSimd folds the first pc channels.
                if pc:
                    nc.gpsimd.tensor_add(
                        f3[:, 0:pc, :], x3[:, 0:pc, 0:100], x3[:, 0:pc, 100:200]
                    )
                if pending is not None:
                    finish(*pending)
                # L1b: DVE folds the rest (2x packed TT).
                if pc < w:
                    nc.vector.tensor_add(
                        f3[:, pc:w, :], x3[:, pc:w, 0:100], x3[:, pc:w, 100:200]
                    )
                # L2: 100 -> 50 (DVE, packed; waits on both L1 halves).
                gt = gp.tile([128, max_w * 50], mybir.dt.bfloat16, tag="gt")
                g3 = gt[:, : w * 50].rearrange("p (g v) -> p g v", v=50)
                nc.vector.tensor_add(g3, f3[:, :, 0:50], f3[:, :, 50:100])
                pending = (t, w, g3)
            finish(*pending)
            nc.sync.dma_start(out=out[0:128, :], in_=osb)
    nc.finalize()
    return nc


def _get_nc():
    global _NC_CACHE
    if _NC_CACHE is None:
        _NC_CACHE = _build_nc()
    return _NC_CACHE


def _norms_from_partials(partials):
    """partials: [128, N_TILES] per-core -> per-row norms [228]."""
    nsq = np.zeros(ROWS_PER_CORE, dtype=np.float64)
    for t, (r0, w, _) in enumerate(TILE_PLAN):
        b = C // w
        ps = partials[:, t].astype(np.float64).reshape(w // 2, b).sum(axis=1)
        nsq[r0 : r0 + w // 2] += ps
    return np.sqrt(nsq) / float(HW)


def _zero_mask():
    mask = np.ones((M, M), dtype=np.float64)
    for i in range(NUM_SAMPLES):
        r = (1 + i) * NUM_NODES
        for c in range(1, M):
            if c % NUM_NODES != 0 and (c - 1) // NUM_NODES != i:
                mask[r, c] = 0.0
    return mask


def _pattern_mixer_np(mat, sigma, lin_w, lin_b, mixed_mat):
    mat = np.asarray(mat, np.float64)            # [5, 7, 7]
    sigma = np.asarray(sigma, np.float64)        # [4, 5, 1]
    lin_w = np.asarray(lin_w, np.float64)        # [4, 5]
    lin_b = np.asarray(lin_b, np.float64)        # [4]
    mixed_mat = np.asarray(mixed_mat, np.float64)  # [4, 57, 57]

    T2 = 2 * NUM_FRAME - 1  # 15
    dist = np.abs(np.arange(T2, dtype=np.float64) - (NUM_FRAME - 1))
    te = (1.0 / (np.sqrt(2.0 * np.pi) * sigma)) * np.exp(
        -(dist**2) / (2.0 * sigma**2)
    )  # [4, 5, 15]
    ce = 1.0 / (1.0 + np.exp(-te))
    mixed = (
        np.einsum("hbt,bnm,hb->hntm", ce, mat, lin_w)
        + lin_b[:, None, None, None]
    )
    mixed = np.maximum(mixed, 0.0).reshape(NUM_MIXED, NUM_NODES, T2 * NUM_NODES)
    blocks = [
        mixed[
            :,
            :,
            NUM_NODES * (NUM_SAMPLES - 1 - i) : NUM_NODES * (2 * NUM_SAMPLES - 1 - i),
        ]
        for i in range(NUM_SAMPLES)
    ]
    add_block = np.concatenate(blocks, axis=1)  # [4, 56, 56]
    mm = mixed_mat.copy()
    mm[:, 1:, 1:] += add_block
    mm *= _zero_mask()[None]
    deg = np.maximum(mm.sum(axis=2), 1.0) ** -0.5  # [4, 57]
    return (deg[:, :, None] * mm * deg[:, None, :]).astype(np.float32)


def kernel(mat, x, sigma, lin_w, lin_b, mixed_mat, alpha):
    global LAST_RESULT
    xf = np.asarray(x, dtype=np.float32).reshape(ROWS_TOTAL, C, HW)
    xs = np.zeros((ROWS_TOTAL, C, WP), dtype=ml_dtypes.bfloat16)
    xs[:, :, :HW] = xf.astype(ml_dtypes.bfloat16)
    xs = xs.reshape(ROWS_TOTAL, CWP)
    in_maps = [
        {"x": xs[i * ROWS_PER_CORE : (i + 1) * ROWS_PER_CORE]} for i in range(N_CORES)
    ]
    nc = _get_nc()
    res = run_bass_kernel_spmd(nc, in_maps, core_ids=list(range(N_CORES)))
    LAST_RESULT = res
    norms = np.concatenate([_norms_from_partials(r["out"]) for r in res.results])
    nrm = norms.reshape(B, M).astype(np.float32)

    mp = _pattern_mixer_np(mat, sigma, lin_w, lin_b, mixed_mat)  # [4, 57, 57] f32
    alpha = np.asarray(alpha, np.float32).reshape(1, NUM_MIXED, 1, 1)
    out = mp[None] + alpha * nrm[:, None, None, :]
    return np.ascontiguousarray(out.astype(np.float32))


# revision 20
# speedup vs baseline: 1.5857x; 1.0002x over previous
"""Trainium2 Bass kernel for nn_Amplified_PatternMixer.

Computation:
  out[b, h, m1, m2] = mixed_pattern[h, m1, m2] + alpha[h] * nrm[b, m2]
where
  nrm[b, m] = || mean_{hw}(x[b*57+m, :, h, w]) ||_2   over channels
  mixed_pattern = tiny 57x57 graph-normalized pattern (from 5x7x7 params).

The memory-bound part (streaming x: [1824, 256, 14, 14]) runs on 8
NeuronCores, data-parallel over rows (228 rows/core).

Optimization history / HW facts (measured on this trn2):
  * f32 stream was DMA-bound at ~130-156us (45.8 MB/core, ~23-27 B/ns
    per SDMA engine x16).
  * Upload dtype is ours to choose: host casts x to bf16 (RNE).  The
    pooled-mean + channel-norm averages away the 0.4% per-element
    quantization noise (measured end-to-end rel err ~2e-4 vs the 2e-2
    gate).  Halves DMA bytes -> ~62us floor.
  * DVE TENSOR_REDUCE only has a 1x uop (1 elem/cycle/lane at ANY
    dtype; bf16 run measured 1.064 ns/elem, same as f32), so a plain
    reduce is a 93us floor -> restructure as a TT tree: bf16
    tensor_tensor DOES hit the 2x_1P packed mode (2 outs = 4 ins per
    cycle, measured 0.55 cyc/out), but only while every src/dst is
    2-byte, step +-1 and 4-byte aligned.  196 = 4*49 breaks alignment
    at the second level, so the host pads each channel 196 -> 200
    zeros (+2% DMA): L1 200->100 and L2 100->50 both stay packed, the
    final 50-wide reduce runs at 1x.  Measured 4626 ns per 32-channel
    tile = 145 ns/channel-column -> ~66us DVE for the full core load.
  * GpSimd tensor_add (bf16) measured 175 ns/channel-column for the L1
    halve -- ~3x slower than DVE but it is an otherwise-idle engine, so
    it folds a fraction of each tile's channels to pull DVE below the
    DMA floor.
  * ScalarE ACTIVATE+READ_ACCUMULATOR costs ~740 ns per accumulate
    instruction pair regardless of size -> only used for the tiny
    per-tile Square(cs) -> sum-of-squares partials.
  * fp8 would halve DMA again but no engine reduces fp8 faster than
    1 elem/cycle (DVE fp8 TT measured 1.04 cyc/out, no packing; PE
    LDW+MM measured 104+165 ns per 128x128 block = way under the
    needed rate), so fp8 upload is compute-dead.  bf16 is optimal.

Loads are issued via HWDGE (nc.sync/nc.scalar); SWDGE (gpsimd) was
measured to collapse DMA rate while DVE runs (shared SBUF port).
"""

import ml_dtypes
import numpy as np

import concourse.bacc as bacc
import concourse.mybir as mybir
import concourse.tile as tile
from concourse.bass_utils import run_bass_kernel_spmd

# Problem constants (hardcoded; kernel.py must be self-contained).
NUM_BASIC = 5
NUM_MIXED = 4
NUM_FRAME = 8
NUM_NODES = 7
NUM_SAMPLES = 8
M = 1 + NUM_NODES * NUM_FRAME  # 57

N_CORES = 8
B = 32
C = 256
HW = 196   # 14*14 (true)
WP = 200   # host-padded channel width (4 zeros) to keep the TT tree
           # 4B-aligned at every level: 200 -> 100 -> 50
ROWS_TOTAL = B * M          # 1824
ROWS_PER_CORE = ROWS_TOTAL // N_CORES  # 228
CWP = C * WP                # 51200 bf16 per row (padded)

# (row_start, w, pool_ch): tile covers rows row_start..row_start+w//2 as
# 128 pieces of w channels (w/2 rows x 256/w pieces); per-partition DMA
# run = w*200*2 B, one descriptor per partition.  pool_ch leading
# channels of each tile are L1-folded on GpSimd instead of DVE.
# Ramp lets DVE start early; 64-wide body maximizes descriptor size
# (25.6 KB) for DMA efficiency; taper keeps the drain short.
_WIDTHS = [4, 8, 16, 32, 64, 64, 64, 64, 64, 32, 16, 16, 8, 4]
assert sum(_WIDTHS) == 2 * ROWS_PER_CORE
assert all(C % w == 0 for w in _WIDTHS)


def _pool_share(w):
    # GpSimd folds a slice of big-tile channels.  POOL's tensor_add
    # contends with DVE's 2x-packed TENSOR_TENSOR for the shared SBUF
    # port (measured 2-4x TT slowdown when overlapped) but leaves the
    # 1x TENSOR_REDUCE untouched, so the share is sized to fit inside
    # the previous tile's reduce window (~3.6us for w=64): 20 channels
    # x 175ns = 3.5us.
    return (5 * w // 16) if w >= 32 else 0


TILE_PLAN = []
_r = 0
for _w in _WIDTHS:
    TILE_PLAN.append((_r, _w, _pool_share(_w)))
    _r += _w // 2
N_TILES = len(TILE_PLAN)

LAST_RESULT = None
_NC_CACHE = None


def _build_nc(plan=TILE_PLAN, bufs=4, rings=("sync", "scalar")):
    nc = bacc.Bacc(None)
    x = nc.declare_dram_parameter(
        "x", [ROWS_PER_CORE, CWP], mybir.dt.bfloat16, isOutput=False
    )
    out = nc.declare_dram_parameter(
        "out", [128, len(plan)], mybir.dt.float32, isOutput=True
    )
    max_w = max(w for _, w, _ in plan)
    with tile.TileContext(nc) as tc:
        with (
            tc.tile_pool(name="xt_pool", bufs=bufs) as xp,
            tc.tile_pool(name="ft_pool", bufs=3) as fp,
            tc.tile_pool(name="gt_pool", bufs=2) as gp,
            tc.tile_pool(name="acc_pool", bufs=2) as accp,
            tc.tile_pool(name="res_pool", bufs=1) as resp,
        ):
            osb = resp.tile([128, len(plan)], mybir.dt.float32, tag="osb")

            def finish(t, w, g3):
                # Final 50-wide reduce (1x) -> per-channel sums, then
                # Square + per-partition sum on the scalar engine.
                cs = accp.tile([128, max_w], mybir.dt.bfloat16, tag="cs")
                with nc.allow_low_precision("bf16 sums; fp32 internal accum"):
                    nc.vector.reduce_sum(cs[:, :w], g3, axis=mybir.AxisListType.X)
                tr = accp.tile([128, max_w], mybir.dt.float32, tag="tr")
                nc.scalar.activation(
                    tr[:, :w],
                    cs[:, :w],
                    mybir.ActivationFunctionType.Square,
                    accum_out=osb[:, t : t + 1],
                )

            # Software-pipelined by one tile, with the DVE queue ordered
            # [reduce(t-1), L1b(t), L2(t)]: GpSimd's L1a(t) fold starts
            # when DMA(t) lands, which is when DVE starts the (port-
            # light, contention-immune) reduce of t-1 -- so POOL never
            # overlaps a packed TT, and DVE never stalls on the POOL
            # semaphore (L2(t) issues after both L1 halves).
            pdum = resp.tile([1, 4], mybir.dt.bfloat16, tag="pdum")
            pending = None
            for t, (r0, w, pc) in enumerate(plan):
                b = C // w          # pieces per row
                f = w * WP          # bf16 elems per partition
                xt = xp.tile([128, max_w * WP], mybir.dt.bfloat16, tag="xt")
                eng = getattr(nc, rings[t % len(rings)])
                src = x[r0 : r0 + w // 2, :].rearrange("a (b f) -> (a b) f", b=b)
                eng.dma_start(out=xt[:, :f], in_=src)

                x3 = xt[:, :f].rearrange("p (g v) -> p g v", v=WP)
                ft = fp.tile([128, max_w * 100], mybir.dt.bfloat16, tag="ft")
                f3 = ft[:, : w * 100].rearrange("p (g v) -> p g v", v=100)
                # L1a: GpSimd folds the first pc channels.  The dummy
                # read of gt(t-1) gates the fold on L2(t-1) completion,
                # phase-locking POOL to DVE's (contention-immune) reduce
                # window -- an unconstrained POOL start drifts onto DVE's
                # packed TTs and halves their throughput (shared SBUF
                # port, measured 2-4x TT inflation).
                if pc:
                    if pending is not None:
                        gprev = pending[3]
                        nc.gpsimd.tensor_add(
                            pdum[0:1, 0:4], gprev[0:1, 0:4], gprev[0:1, 0:4]
                        )
                    nc.gpsimd.tensor_add(
                        f3[:, 0:pc, :], x3[:, 0:pc, 0:100], x3[:, 0:pc, 100:200]
                    )
                if pending is not None:
                    finish(*pending[:3])
                # L1b: DVE folds the rest (2x packed TT).
                if pc < w:
                    nc.vector.tensor_add(
                        f3[:, pc:w, :], x3[:, pc:w, 0:100], x3[:, pc:w, 100:200]
                    )
                # L2: 100 -> 50 (DVE, packed; waits on both L1 halves).
                gt = gp.tile([128, max_w * 50], mybir.dt.bfloat16, tag="gt")
                g3 = gt[:, : w * 50].rearrange("p (g v) -> p g v", v=50)
                nc.vector.tensor_add(g3, f3[:, :, 0:50], f3[:, :, 50:100])
                pending = (t, w, g3, gt)
            finish(*pending[:3])
            nc.sync.dma_start(out=out[0:128, :], in_=osb)
    nc.finalize()
    return nc


def _get_nc():
    global _NC_CACHE
    if _NC_CACHE is None:
        _NC_CACHE = _build_nc()
    return _NC_CACHE


def _norms_from_partials(partials):
    """partials: [128, N_TILES] per-core -> per-row norms [228]."""
    nsq = np.zeros(ROWS_PER_CORE, dtype=np.float64)
    for t, (r0, w, _) in enumerate(TILE_PLAN):
        b = C // w
        ps = partials[:, t].astype(np.float64).reshape(w // 2, b).sum(axis=1)
        nsq[r0 : r0 + w // 2] += ps
    return np.sqrt(nsq) / float(HW)


def _zero_mask():
    mask = np.ones((M, M), dtype=np.float64)
    for i in range(NUM_SAMPLES):
        r = (1 + i) * NUM_NODES
        for c in range(1, M):
            if c % NUM_NODES != 0 and (c - 1) // NUM_NODES != i:
                mask[r, c] = 0.0
    return mask


def _pattern_mixer_np(mat, sigma, lin_w, lin_b, mixed_mat):
    mat = np.asarray(mat, np.float64)            # [5, 7, 7]
    sigma = np.asarray(sigma, np.float64)        # [4, 5, 1]
    lin_w = np.asarray(lin_w, np.float64)        # [4, 5]
    lin_b = np.asarray(lin_b, np.float64)        # [4]
    mixed_mat = np.asarray(mixed_mat, np.float64)  # [4, 57, 57]

    T2 = 2 * NUM_FRAME - 1  # 15
    dist = np.abs(np.arange(T2, dtype=np.float64) - (NUM_FRAME - 1))
    te = (1.0 / (np.sqrt(2.0 * np.pi) * sigma)) * np.exp(
        -(dist**2) / (2.0 * sigma**2)
    )  # [4, 5, 15]
    ce = 1.0 / (1.0 + np.exp(-te))
    mixed = (
        np.einsum("hbt,bnm,hb->hntm", ce, mat, lin_w)
        + lin_b[:, None, None, None]
    )
    mixed = np.maximum(mixed, 0.0).reshape(NUM_MIXED, NUM_NODES, T2 * NUM_NODES)
    blocks = [
        mixed[
            :,
            :,
            NUM_NODES * (NUM_SAMPLES - 1 - i) : NUM_NODES * (2 * NUM_SAMPLES - 1 - i),
        ]
        for i in range(NUM_SAMPLES)
    ]
    add_block = np.concatenate(blocks, axis=1)  # [4, 56, 56]
    mm = mixed_mat.copy()
    mm[:, 1:, 1:] += add_block
    mm *= _zero_mask()[None]
    deg = np.maximum(mm.sum(axis=2), 1.0) ** -0.5  # [4, 57]
    return (deg[:, :, None] * mm * deg[:, None, :]).astype(np.float32)


def kernel(mat, x, sigma, lin_w, lin_b, mixed_mat, alpha):
    global LAST_RESULT
    xf = np.asarray(x, dtype=np.float32).reshape(ROWS_TOTAL, C, HW)
    xs = np.zeros((ROWS_TOTAL, C, WP), dtype=ml_dtypes.bfloat16)
    xs[:, :, :HW] = xf.astype(ml_dtypes.bfloat16)
    xs = xs.reshape(ROWS_TOTAL, CWP)
    in_maps = [
        {"x": xs[i * ROWS_PER_CORE : (i + 1) * ROWS_PER_CORE]} for i in range(N_CORES)
    ]
    nc = _get_nc()
    res = run_bass_kernel_spmd(nc, in_maps, core_ids=list(range(N_CORES)))
    LAST_RESULT = res
    norms = np.concatenate([_norms_from_partials(r["out"]) for r in res.results])
    nrm = norms.reshape(B, M).astype(np.float32)

    mp = _pattern_mixer_np(mat, sigma, lin_w, lin_b, mixed_mat)  # [4, 57, 57] f32
    alpha = np.asarray(alpha, np.float32).reshape(1, NUM_MIXED, 1, 1)
    out = mp[None] + alpha * nrm[:, None, None, :]
    return np.ascontiguousarray(out.astype(np.float32))
